# revision 115
# baseline (speedup 1.0000x reference)
"""BitSelfAttention (relative_key_query position bias) on 8 trn2 cores.

Sharding: core c -> batch b=c//2, head-group g=c%2 (8 heads of 64 dims).
Per core: q/k/v projections for its 512 output dims, then per-head
attention with the Toeplitz relative-position bias realized via a DRAM
round-trip (skewed access patterns) for the Eq/Ek tables.

Layout: scores are computed TRANSPOSED (scoresT[r, l]) so that
 - softmax denominators come free as an extra ones-column in the PV matmul
 - probs never need transposing for PV (expT blocks are the PV stationary)
 - rel_k reads from DRAM are contiguous; rel_q arrives via xbar DMA-transpose
   of a column-reversed Eq table (reversal folded into the host-side deT flip).

v2 changes vs v1 (cost-model exec 514us -> 358us, HW-verified correct):
 - Ek table stored fp8e4 in DRAM (halves its round-trip HBM traffic); added
   into scores with a second identity-matmul in fp8 (exact fp8 identity).
 - attention mask folded into the exp's per-partition bias operand
   (activation computes exp(sc/8 + mask[r])); q/k projection biases folded
   into the PSUM->SBUF copies; band tail matmul trimmed to the exact
   1152-wide window.
 - software-pipelined emission at depth 3: bands(h) | table reads(h-1) |
   scores+PV(h-2), so each head's DRAM round-trip latency is covered by
   later heads' band work.  All PSUM tiles are 1-bank [128,512]f32:
   band ring 4 bufs, score ring 3, ctx 1.
 - PV tail: per-lt ctx+den leave PSUM via one cheap DVE copy, one
   reciprocal per head on the gathered denominators, then 8 muls; output
   written per head-PAIR (512B rows, full DMA rate) from the Pool queue --
   no reciprocal head-of-line blocking in DVE, no output blocking on SP.
 - stage copies split ~half/half between ACT and DVE; DMA issue spread
   across SP (transpose + ek reads, loads) and ACT (stage writes) so no
   compute queue waits on a DMA it doesn't feed; expt ring holds two
   heads to decouple head handoffs.
"""
import math
from contextlib import ExitStack

import numpy as np

import concourse.bass as bass
import concourse.bacc as bacc
import concourse.tile as tile
from concourse import mybir
from concourse.bass_utils import run_bass_kernel_spmd

B, S, D, H = 4, 1024, 1024, 16
HD = 64
E = 512          # output dims per core (8 heads)
NHC = 8          # heads per core
WD = 2176        # scratch DRAM row width (896 + 1280)
F32 = mybir.dt.float32
BF16 = mybir.dt.bfloat16
FP8 = mybir.dt.float8e4
IDENT = mybir.ActivationFunctionType.Identity
EXP = mybir.ActivationFunctionType.Exp


def build_nc():
    nc = bacc.Bacc()
    hT = nc.declare_dram_parameter("hT", [D, S], BF16, isOutput=False)
    wqT = nc.declare_dram_parameter("wqT", [D, E], BF16, isOutput=False)
    wkT = nc.declare_dram_parameter("wkT", [D, E], BF16, isOutput=False)
    wvT = nc.declare_dram_parameter("wvT", [D, E], BF16, isOutput=False)
    bqc = nc.declare_dram_parameter("bqc", [128, 4], F32, isOutput=False)
    bkc = nc.declare_dram_parameter("bkc", [128, 4], F32, isOutput=False)
    bvr = nc.declare_dram_parameter("bvr", [1, E], BF16, isOutput=False)
    # deT duplicated onto partitions 64..127 so odd heads' K=64 matmuls
    # use lhsT and rhs at the same base partition.
    deTR = nc.declare_dram_parameter("deTR", [128, WD], BF16, isOutput=False)
    deTP = nc.declare_dram_parameter("deTP", [128, WD], BF16, isOutput=False)
    maskc = nc.declare_dram_parameter("maskc", [128, 8], F32, isOutput=False)
    ident = nc.declare_dram_parameter("ident", [128, 128], BF16, isOutput=False)
    out_t = nc.declare_dram_parameter("out", [S, E], F32, isOutput=True)

    # DRAM scratch, fresh per head (no WAR fan-in on reuse)
    eqr = [nc.dram_tensor(f"eqr{i}", [S, WD], BF16) for i in range(8)]
    ekd = [nc.dram_tensor(f"ekd{i}", [S, WD], FP8) for i in range(8)]

    ctx = ExitStack()
    with ctx:
        tc = ctx.enter_context(tile.TileContext(nc))
        consts = ctx.enter_context(tc.tile_pool(name="consts", bufs=1))
        # PSUM budget (8 banks), all tiles 1 bank ([128,512] f32):
        # bands ring 4 + scores ring 3 + ctx 1
        pband = ctx.enter_context(tc.tile_pool(name="pband", bufs=3, space="PSUM"))
        psc = ctx.enter_context(tc.tile_pool(name="psc", bufs=3, space="PSUM"))
        pctx = ctx.enter_context(tc.tile_pool(name="pctx", bufs=2, space="PSUM"))
        band_sb = ctx.enter_context(tc.tile_pool(name="band_sb", bufs=2))
        rel_pool = ctx.enter_context(tc.tile_pool(name="rel_pool", bufs=14))
        ekt_pool = ctx.enter_context(tc.tile_pool(name="ekt_pool", bufs=10))
        expt_pool = ctx.enter_context(tc.tile_pool(name="expt_pool", bufs=16))
        small = ctx.enter_context(tc.tile_pool(name="small", bufs=4))

        # ---- load inputs to SBUF (spread across issue queues); ht and wq
        # interleaved so the first projection group can start earliest ----
        ht_sb = []
        wq_sb, wk_sb, wv_sb = [], [], []
        for kt in range(8):
            t = consts.tile([128, S], BF16, name=f"ht{kt}")
            nc.sync.dma_start(out=t, in_=hT[kt * 128:(kt + 1) * 128, :])
            ht_sb.append(t)
            t = consts.tile([128, E], BF16, name=f"wq{kt}")
            nc.sync.dma_start(out=t, in_=wqT[kt * 128:(kt + 1) * 128, :])
            wq_sb.append(t)
        for (dst, srct, nm, eng) in ((wk_sb, wkT, "wk", nc.scalar),
                                     (wv_sb, wvT, "wv", nc.scalar)):
            for kt in range(8):
                t = consts.tile([128, E], BF16, name=f"{nm}{kt}")
                eng.dma_start(out=t, in_=srct[kt * 128:(kt + 1) * 128, :])
                dst.append(t)
        detr_sb = consts.tile([128, WD], BF16, name="detr_sb")
        nc.gpsimd.dma_start(out=detr_sb, in_=deTR[:, :])
        detp_sb = consts.tile([128, WD], BF16, name="detp_sb")
        nc.gpsimd.dma_start(out=detp_sb, in_=deTP[:, :])
        mask_sb = consts.tile([128, 8], F32, name="mask_sb")
        nc.gpsimd.dma_start(out=mask_sb, in_=maskc[:, :])
        id_sb = consts.tile([128, 128], BF16, name="id_sb")
        nc.gpsimd.dma_start(out=id_sb, in_=ident[:, :])
        bqc_sb = consts.tile([128, 4], F32, name="bqc_sb")
        nc.gpsimd.dma_start(out=bqc_sb, in_=bqc[:, :])
        bkc_sb = consts.tile([128, 4], F32, name="bkc_sb")
        nc.gpsimd.dma_start(out=bkc_sb, in_=bkc[:, :])
        bv_sb = consts.tile([1, E], BF16, name="bv_sb")
        nc.gpsimd.dma_start(out=bv_sb, in_=bvr[:, :])
        ones_sb = consts.tile([1, E], BF16, name="ones_sb")
        nc.vector.memset(ones_sb, 1.0)
        idf8_sb = consts.tile([128, 128], FP8, name="idf8_sb")
        nc.vector.tensor_copy(idf8_sb, id_sb)

        # ---- projections (bias folded into the PSUM->SBUF copy) ----
        qT_sb = [consts.tile([128, S], BF16, name=f"qT{et}") for et in range(4)]
        kT_sb = [consts.tile([128, S], BF16, name=f"kT{et}") for et in range(4)]
        v_sb = [consts.tile([128, 8, 65], BF16, name=f"v{st}") for st in range(8)]

        def proj_qk_steps(et):
            for ns in range(2):
                for (w_sb, bc_sb, dstl) in ((wq_sb, bqc_sb, qT_sb), (wk_sb, bkc_sb, kT_sb)):
                    psv = pband.tile([128, 512], F32, name="ps_proj", tag="bA")
                    for kt in range(8):
                        nc.tensor.matmul(
                            psv, w_sb[kt][:, et * 128:(et + 1) * 128],
                            ht_sb[kt][:, ns * 512:(ns + 1) * 512],
                            start=(kt == 0), stop=(kt == 7))
                    dst = dstl[et][:, ns * 512:(ns + 1) * 512]
                    if (et + ns) % 2 == 1:
                        nc.vector.tensor_scalar_add(dst, psv, bc_sb[:, et:et + 1])
                    else:
                        nc.scalar.activation(out=dst, in_=psv, func=IDENT,
                                             bias=bc_sb[:, et:et + 1])
                    yield

        def proj_v_steps(st0, st1):
            # v: natural [S, E] as 8 stile x [128, 8, 65] bf16; col 64 = ones
            for st in range(st0, st1):
                psv = pband.tile([128, 512], F32, name="ps_proj", tag="bA")
                for kt in range(8):
                    nc.tensor.matmul(
                        psv, ht_sb[kt][:, st * 128:(st + 1) * 128],
                        wv_sb[kt], start=(kt == 0), stop=False)
                nc.tensor.matmul(psv, ones_sb[0:1, 0:128], bv_sb,
                                 start=False, stop=True)
                nc.vector.tensor_copy(v_sb[st][:, :, 0:64], psv.rearrange("p (h e) -> p h e", h=8))
                nc.vector.memset(v_sb[st][:, :, 64:65], 1.0)
                yield



        # ---- per-head attention, software-pipelined: bands(h) cover the
        # DRAM round-trip latency of head h-1's tables ----
        def band_steps(h):
            """Generator of 16 band steps + 2 stage-write steps for head h."""
            et, po = h // 2, 64 * (h % 2)
            eq_stage = band_sb.tile([128, 8, 1152], BF16, name="eq_stage", tag="eq_stage")
            ek_stage = band_sb.tile([128, 8, 1152], FP8, name="ek_stage", tag="ek_stage")

            def band(lhs, de_sb, stage, idx, on_act):
                base = 896 - 128 * idx
                b0 = pband.tile([128, 512], F32, name="b0", tag="bA")
                b1 = pband.tile([128, 512], F32, name="b1", tag="bA")
                b2 = pband.tile([128, 512], F32, name="b2", tag="bA")
                nc.tensor.matmul(b0, lhs, de_sb[po:po + 64, base:base + 512],
                                 start=True, stop=True)
                nc.tensor.matmul(b1, lhs, de_sb[po:po + 64, base + 512:base + 1024],
                                 start=True, stop=True)
                nc.tensor.matmul(b2[:, 0:128], lhs,
                                 de_sb[po:po + 64, base + 1024:base + 1152],
                                 start=True, stop=True)
                if on_act:
                    nc.scalar.copy(stage[:, idx, 0:512], b0)
                    nc.vector.tensor_copy(stage[:, idx, 512:1024], b1)
                    nc.scalar.copy(stage[:, idx, 1024:1152], b2[:, 0:128])
                else:
                    nc.vector.tensor_copy(stage[:, idx, 0:512], b0)
                    nc.scalar.copy(stage[:, idx, 512:1024], b1)
                    nc.vector.tensor_copy(stage[:, idx, 1024:1152], b2[:, 0:128])

            # Eq first: its transpose-read is the latency-critical input
            # (feeds ideq); Ek feeds the last matmul of each group, so its
            # table can land half a head later.
            for i in range(8):
                # ACT takes 3 of 8 copies per table, DVE the other 5
                band(qT_sb[et][po:po + 64, i * 128:(i + 1) * 128], detr_sb,
                     eq_stage, i, on_act=(i % 8 < 3))
                yield
            nc.scalar.dma_start(
                out=bass.AP(tensor=eqr[h], offset=896,
                            ap=[[WD, 128], [128 * WD - 128, 8], [1, 1152]]),
                in_=eq_stage)
            yield
            for i in range(8):
                band(kT_sb[et][po:po + 64, i * 128:(i + 1) * 128], detp_sb,
                     ek_stage, i, on_act=(i % 8 < 3))
                yield
            nc.gpsimd.dma_start(
                out=bass.AP(tensor=ekd[h], offset=896,
                            ap=[[WD, 128], [128 * WD - 128, 8], [1, 1152]]),
                in_=ek_stage)
            yield

        def emit_reads(h):
            # critical transposes issue ahead of the slack-side ek reads
            rels, ekts = [], []
            for rt in range(8):
                rel = rel_pool.tile([128, S], BF16, name="rel", tag="rel")
                nc.sync.dma_start_transpose(
                    out=rel,
                    in_=bass.AP(tensor=eqr[h], offset=1023 + rt * 128,
                                ap=[[WD - 1, 1024], [1, 128]]))
                rels.append(rel)
            for rt in range(8):
                ekt = ekt_pool.tile([128, S], FP8, name="ekt", tag="ekt")
                nc.sync.dma_start(
                    out=ekt,
                    in_=bass.AP(tensor=ekd[h], offset=(WD - 1) * rt * 128 + 1023,
                                ap=[[WD - 1, 128], [1, 1024]]))
                ekts.append(ekt)
            return rels, ekts

        out16_store = {}

        def score_pv_steps(h, reads):
            """Generator: 8 score steps then 8 PV steps for head h."""
            et, po = h // 2, 64 * (h % 2)
            rels, ekts = reads
            expt = []
            for rt in range(8):
                r0 = rt * 128
                rel, ekt = rels[rt], ekts[rt]
                sc = [psc.tile([128, 512], F32, name="sc", tag="sc")
                      for _ in range(2)]
                for nh in range(2):
                    nc.tensor.matmul(
                        sc[nh],
                        kT_sb[et][po:po + 64, r0:r0 + 128],
                        qT_sb[et][po:po + 64, nh * 512:(nh + 1) * 512],
                        start=True, stop=False)
                for nh in range(2):
                    nc.tensor.matmul(
                        sc[nh], id_sb, rel[:, nh * 512:(nh + 1) * 512],
                        start=False, stop=False)
                for nh in range(2):
                    nc.tensor.matmul(
                        sc[nh], idf8_sb, ekt[:, nh * 512:(nh + 1) * 512],
                        start=False, stop=True)
                ex = expt_pool.tile([128, S], BF16, name="ex", tag="ex")
                for nh in range(2):
                    nc.scalar.activation(out=ex[:, nh * 512:(nh + 1) * 512],
                                         in_=sc[nh], func=EXP,
                                         scale=1.0 / math.sqrt(HD),
                                         bias=mask_sb[:, rt:rt + 1])
                expt.append(ex)
                yield
            ctxden = small.tile([128, 8, 65], F32, name="ctxden", tag="ctxden",
                                bufs=2)
            if h % 2 == 0:
                out16_store[0] = small.tile([128, 8, 128], F32, name="out16",
                                            tag="out16", bufs=1)
            out16 = out16_store[0]
            oc = (h % 2) * 64
            for lt in range(8):
                cx = pctx.tile([128, 65], F32, name="cx", tag="cx")
                for rt in range(8):
                    nc.tensor.matmul(cx, expt[rt][:, lt * 128:(lt + 1) * 128],
                                     v_sb[rt][:, h, :],
                                     start=(rt == 0), stop=(rt == 7))
                # single fast copy releases the PSUM bank; divide later
                nc.vector.tensor_copy(ctxden[:, lt, :], cx)
                yield
            rc8 = small.tile([128, 8], F32, name="rc8", tag="rc8", bufs=2)
            nc.vector.reciprocal(rc8, ctxden[:, :, 64])
            for lt in range(8):
                nc.vector.tensor_scalar_mul(
                    out16[:, lt, oc:oc + 64], ctxden[:, lt, 0:64],
                    rc8[:, lt:lt + 1])
            if h % 2 == 1:
                # one write per head pair: 512B contiguous rows, full DMA rate
                nc.gpsimd.dma_start(
                    out=bass.AP(tensor=out_t, offset=(h // 2) * 128,
                                ap=[[E, 128], [E * 128, 8], [1, 128]]),
                    in_=out16)
            yield

        # depth-3 pipeline, interleaved emission so engine priority order
        # alternates: bands(h) | reads(h-1) | scores+PV(h-2)
        def exhaust(g):
            if g is not None:
                for _ in g:
                    pass

        exhaust(proj_qk_steps(0))
        extras = {0: [proj_qk_steps(1), proj_v_steps(0, 4)],
                  1: [proj_qk_steps(2), proj_v_steps(4, 8)],
                  2: [proj_qk_steps(3)]}
        reads_of = {}
        for h in range(NHC + 2):
            if 1 <= h <= NHC:
                reads_of[h - 1] = emit_reads(h - 1)
            gens = list(extras.get(h, []))
            if h < NHC:
                gens.append(band_steps(h))
            if h >= 2:
                gens.append(score_pv_steps(h - 2, reads_of.pop(h - 2)))
            while gens:
                for g in list(gens):
                    try:
                        next(g)
                    except StopIteration:
                        gens.remove(g)

    nc.compile()
    return nc


_NC_CACHE = {}
LAST_RESULT = None


def kernel(hidden_states, attention_mask, Wq, bq, Wk, bk, Wv, bv, dist_emb):
    hidden_states = np.asarray(hidden_states, np.float32)
    attention_mask = np.asarray(attention_mask, np.float32)
    Wq, bq = np.asarray(Wq, np.float32), np.asarray(bq, np.float32)
    Wk, bk = np.asarray(Wk, np.float32), np.asarray(bk, np.float32)
    Wv, bv = np.asarray(Wv, np.float32), np.asarray(bv, np.float32)
    dist_emb = np.asarray(dist_emb, np.float32)
    bf = mybir.dt.np(BF16)

    deT = dist_emb.T  # [64, 2047]
    deTP = np.zeros((128, WD), np.float32)
    deTP[0:64, :2047] = deT
    deTP[64:128, :2047] = deT
    deTR = np.zeros((128, WD), np.float32)
    deTR[0:64, :2047] = deT[:, ::-1]
    deTR[64:128, :2047] = deT[:, ::-1]
    ident = np.eye(128).astype(bf)

    if "nc" not in _NC_CACHE:
        _NC_CACHE["nc"] = build_nc()
    nc = _NC_CACHE["nc"]

    in_maps = []
    for c in range(8):
        b, g = c // 2, c % 2
        esl = slice(g * E, (g + 1) * E)
        mk = attention_mask[b, 0, 0, :].astype(np.float32)
        in_maps.append({
            "hT": np.ascontiguousarray(hidden_states[b].T).astype(bf),
            "wqT": np.ascontiguousarray(Wq[esl, :].T).astype(bf),
            "wkT": np.ascontiguousarray(Wk[esl, :].T).astype(bf),
            "wvT": np.ascontiguousarray(Wv[esl, :].T).astype(bf),
            "bqc": np.ascontiguousarray(bq[esl].reshape(4, 128).T),
            "bkc": np.ascontiguousarray(bk[esl].reshape(4, 128).T),
            "bvr": np.ascontiguousarray(bv[esl][None, :]).astype(bf),
            "deTR": deTR.astype(bf), "deTP": deTP.astype(bf),
            "maskc": np.ascontiguousarray(mk.reshape(8, 128).T),
            "ident": ident,
        })
    import os as _os
    res = run_bass_kernel_spmd(nc, in_maps, core_ids=list(range(8)),
                               trace=bool(_os.environ.get("KTRACE")),
                               tmpdir=_os.environ.get("KTRACE_DIR") or None)
    global LAST_RESULT
    LAST_RESULT = res
    out = np.empty((B, S, D), np.float32)
    for c in range(8):
        b, g = c // 2, c % 2
        out[b, :, g * E:(g + 1) * E] = res.results[c]["out"]
    return out


# revision 116
# speedup vs baseline: 1.0976x; 1.0976x over previous
"""BitSelfAttention (relative_key_query position bias) on 8 trn2 cores.

Sharding: core c -> batch b=c//2, head-group g=c%2 (8 heads of 64 dims).
Per core: q/k/v projections for its 512 output dims, then per-head
attention with the Toeplitz relative-position bias realized via a DRAM
round-trip (skewed access patterns) for the Eq/Ek tables.

Layout: scores are computed TRANSPOSED (scoresT[r, l]) so that
 - softmax denominators come free as an extra ones-column in the PV matmul
 - probs never need transposing for PV (expT blocks are the PV stationary)
 - rel_k reads from DRAM are contiguous; rel_q arrives via xbar DMA-transpose
   of a column-reversed Eq table (reversal folded into the host-side deT flip).

v2 changes vs v1 (cost-model exec 514us -> 358us, HW-verified correct):
 - Ek table stored fp8e4 in DRAM (halves its round-trip HBM traffic); added
   into scores with a second identity-matmul in fp8 (exact fp8 identity).
 - attention mask folded into the exp's per-partition bias operand
   (activation computes exp(sc/8 + mask[r])); q/k projection biases folded
   into the PSUM->SBUF copies; band tail matmul trimmed to the exact
   1152-wide window.
 - software-pipelined emission at depth 3: bands(h) | table reads(h-1) |
   scores+PV(h-2), so each head's DRAM round-trip latency is covered by
   later heads' band work.  All PSUM tiles are 1-bank [128,512]f32:
   band ring 4 bufs, score ring 3, ctx 1.
 - PV tail: per-lt ctx+den leave PSUM via one cheap DVE copy, one
   reciprocal per head on the gathered denominators, then 8 muls; output
   written per head-PAIR (512B rows, full DMA rate) from the Pool queue --
   no reciprocal head-of-line blocking in DVE, no output blocking on SP.
 - stage copies split ~half/half between ACT and DVE; DMA issue spread
   across SP (transpose + ek reads, loads) and ACT (stage writes) so no
   compute queue waits on a DMA it doesn't feed; expt ring holds two
   heads to decouple head handoffs.
"""
import math
from contextlib import ExitStack

import numpy as np

import concourse.bass as bass
import concourse.bacc as bacc
import concourse.tile as tile
from concourse import mybir
from concourse.bass_utils import run_bass_kernel_spmd

B, S, D, H = 4, 1024, 1024, 16
HD = 64
E = 512          # output dims per core (8 heads)
NHC = 8          # heads per core
WD = 2176        # scratch DRAM row width (896 + 1280)
F32 = mybir.dt.float32
BF16 = mybir.dt.bfloat16
FP8 = mybir.dt.float8e4
IDENT = mybir.ActivationFunctionType.Identity
EXP = mybir.ActivationFunctionType.Exp


def build_nc():
    nc = bacc.Bacc()
    hT = nc.declare_dram_parameter("hT", [D, S], BF16, isOutput=False)
    wqT = nc.declare_dram_parameter("wqT", [D, E], BF16, isOutput=False)
    wkT = nc.declare_dram_parameter("wkT", [D, E], BF16, isOutput=False)
    wvT = nc.declare_dram_parameter("wvT", [D, E], BF16, isOutput=False)
    bqc = nc.declare_dram_parameter("bqc", [128, 4], F32, isOutput=False)
    bkc = nc.declare_dram_parameter("bkc", [128, 4], F32, isOutput=False)
    bvr = nc.declare_dram_parameter("bvr", [1, E], BF16, isOutput=False)
    # deT duplicated onto partitions 64..127 so odd heads' K=64 matmuls
    # use lhsT and rhs at the same base partition.
    deTR = nc.declare_dram_parameter("deTR", [128, WD], BF16, isOutput=False)
    deTP = nc.declare_dram_parameter("deTP", [128, WD], BF16, isOutput=False)
    maskc = nc.declare_dram_parameter("maskc", [128, 8], F32, isOutput=False)
    ident = nc.declare_dram_parameter("ident", [128, 128], BF16, isOutput=False)
    out_t = nc.declare_dram_parameter("out", [S, E], F32, isOutput=True)

    # DRAM scratch, fresh per head (no WAR fan-in on reuse)
    eqr = [nc.dram_tensor(f"eqr{i}", [S, WD], BF16) for i in range(8)]
    ekd = [nc.dram_tensor(f"ekd{i}", [S, WD], FP8) for i in range(8)]

    ctx = ExitStack()
    with ctx:
        tc = ctx.enter_context(tile.TileContext(nc))
        consts = ctx.enter_context(tc.tile_pool(name="consts", bufs=1))
        # PSUM budget (8 banks), all tiles 1 bank ([128,512] f32):
        # bands ring 4 + scores ring 3 + ctx 1
        pband = ctx.enter_context(tc.tile_pool(name="pband", bufs=3, space="PSUM"))
        psc = ctx.enter_context(tc.tile_pool(name="psc", bufs=3, space="PSUM"))
        pctx = ctx.enter_context(tc.tile_pool(name="pctx", bufs=2, space="PSUM"))
        band_sb = ctx.enter_context(tc.tile_pool(name="band_sb", bufs=2))
        rel_pool = ctx.enter_context(tc.tile_pool(name="rel_pool", bufs=14))
        ekt_pool = ctx.enter_context(tc.tile_pool(name="ekt_pool", bufs=10))
        expt_pool = ctx.enter_context(tc.tile_pool(name="expt_pool", bufs=16))
        small = ctx.enter_context(tc.tile_pool(name="small", bufs=4))

        # ---- load inputs to SBUF (spread across issue queues); ht and wq
        # interleaved so the first projection group can start earliest ----
        ht_sb = []
        wq_sb, wk_sb, wv_sb = [], [], []
        for kt in range(8):
            t = consts.tile([128, S], BF16, name=f"ht{kt}")
            nc.sync.dma_start(out=t, in_=hT[kt * 128:(kt + 1) * 128, :])
            ht_sb.append(t)
            t = consts.tile([128, E], BF16, name=f"wq{kt}")
            nc.sync.dma_start(out=t, in_=wqT[kt * 128:(kt + 1) * 128, :])
            wq_sb.append(t)
        for (dst, srct, nm, eng) in ((wk_sb, wkT, "wk", nc.scalar),
                                     (wv_sb, wvT, "wv", nc.scalar)):
            for kt in range(8):
                t = consts.tile([128, E], BF16, name=f"{nm}{kt}")
                eng.dma_start(out=t, in_=srct[kt * 128:(kt + 1) * 128, :])
                dst.append(t)
        detr_sb = consts.tile([128, WD], BF16, name="detr_sb")
        nc.gpsimd.dma_start(out=detr_sb, in_=deTR[:, :])
        detp_sb = consts.tile([128, WD], BF16, name="detp_sb")
        nc.gpsimd.dma_start(out=detp_sb, in_=deTP[:, :])
        mask_sb = consts.tile([128, 8], F32, name="mask_sb")
        nc.gpsimd.dma_start(out=mask_sb, in_=maskc[:, :])
        id_sb = consts.tile([128, 128], BF16, name="id_sb")
        nc.gpsimd.dma_start(out=id_sb, in_=ident[:, :])
        bqc_sb = consts.tile([128, 4], F32, name="bqc_sb")
        nc.gpsimd.dma_start(out=bqc_sb, in_=bqc[:, :])
        bkc_sb = consts.tile([128, 4], F32, name="bkc_sb")
        nc.gpsimd.dma_start(out=bkc_sb, in_=bkc[:, :])
        bv_sb = consts.tile([1, E], BF16, name="bv_sb")
        nc.gpsimd.dma_start(out=bv_sb, in_=bvr[:, :])
        ones_sb = consts.tile([1, E], BF16, name="ones_sb")
        nc.vector.memset(ones_sb, 1.0)
        idf8_sb = consts.tile([128, 128], FP8, name="idf8_sb")
        nc.vector.tensor_copy(idf8_sb, id_sb)

        # ---- projections (bias folded into the PSUM->SBUF copy) ----
        qT_sb = [consts.tile([128, S], BF16, name=f"qT{et}") for et in range(4)]
        kT_sb = [consts.tile([128, S], BF16, name=f"kT{et}") for et in range(4)]
        v_sb = [consts.tile([128, 8, 65], BF16, name=f"v{st}") for st in range(8)]

        def proj_qk_steps(et):
            for ns in range(2):
                for (w_sb, bc_sb, dstl) in ((wq_sb, bqc_sb, qT_sb), (wk_sb, bkc_sb, kT_sb)):
                    # ctx ring is idle until iteration 2; keep bands' ring free
                    psv = pctx.tile([128, 512], F32, name="ps_proj", tag="cx")
                    for kt in range(8):
                        nc.tensor.matmul(
                            psv, w_sb[kt][:, et * 128:(et + 1) * 128],
                            ht_sb[kt][:, ns * 512:(ns + 1) * 512],
                            start=(kt == 0), stop=(kt == 7))
                    dst = dstl[et][:, ns * 512:(ns + 1) * 512]
                    if (et + ns) % 2 == 1:
                        nc.vector.tensor_scalar_add(dst, psv, bc_sb[:, et:et + 1])
                    else:
                        nc.scalar.activation(out=dst, in_=psv, func=IDENT,
                                             bias=bc_sb[:, et:et + 1])
                    yield

        def proj_v_steps(st0, st1):
            # v: natural [S, E] as 8 stile x [128, 8, 65] bf16; col 64 = ones
            for st in range(st0, st1):
                psv = pctx.tile([128, 512], F32, name="ps_proj", tag="cx")
                for kt in range(8):
                    nc.tensor.matmul(
                        psv, ht_sb[kt][:, st * 128:(st + 1) * 128],
                        wv_sb[kt], start=(kt == 0), stop=False)
                nc.tensor.matmul(psv, ones_sb[0:1, 0:128], bv_sb,
                                 start=False, stop=True)
                nc.vector.tensor_copy(v_sb[st][:, :, 0:64], psv.rearrange("p (h e) -> p h e", h=8))
                nc.vector.memset(v_sb[st][:, :, 64:65], 1.0)
                yield



        # ---- per-head attention, software-pipelined: bands(h) cover the
        # DRAM round-trip latency of head h-1's tables ----
        def band_steps(h):
            """Generator of 16 band steps + 2 stage-write steps for head h."""
            et, po = h // 2, 64 * (h % 2)
            eq_stage = band_sb.tile([128, 8, 1152], BF16, name="eq_stage", tag="eq_stage")
            ek_stage = band_sb.tile([128, 8, 1152], FP8, name="ek_stage", tag="ek_stage")

            def band(lhs, de_sb, stage, idx, on_act):
                base = 896 - 128 * idx
                b0 = pband.tile([128, 512], F32, name="b0", tag="bA")
                b1 = pband.tile([128, 512], F32, name="b1", tag="bA")
                b2 = pband.tile([128, 512], F32, name="b2", tag="bA")
                nc.tensor.matmul(b0, lhs, de_sb[po:po + 64, base:base + 512],
                                 start=True, stop=True)
                nc.tensor.matmul(b1, lhs, de_sb[po:po + 64, base + 512:base + 1024],
                                 start=True, stop=True)
                nc.tensor.matmul(b2[:, 0:128], lhs,
                                 de_sb[po:po + 64, base + 1024:base + 1152],
                                 start=True, stop=True)
                if on_act:
                    nc.scalar.copy(stage[:, idx, 0:512], b0)
                    nc.vector.tensor_copy(stage[:, idx, 512:1024], b1)
                    nc.scalar.copy(stage[:, idx, 1024:1152], b2[:, 0:128])
                else:
                    nc.vector.tensor_copy(stage[:, idx, 0:512], b0)
                    nc.scalar.copy(stage[:, idx, 512:1024], b1)
                    nc.vector.tensor_copy(stage[:, idx, 1024:1152], b2[:, 0:128])

            # Eq first: its transpose-read is the latency-critical input
            # (feeds ideq); Ek feeds the last matmul of each group, so its
            # table can land half a head later.
            for i in range(8):
                # ACT takes 3 of 8 copies per table, DVE the other 5
                band(qT_sb[et][po:po + 64, i * 128:(i + 1) * 128], detr_sb,
                     eq_stage, i, on_act=(i % 8 < 3))
                yield
            nc.scalar.dma_start(
                out=bass.AP(tensor=eqr[h], offset=896,
                            ap=[[WD, 128], [128 * WD - 128, 8], [1, 1152]]),
                in_=eq_stage)
            yield
            for i in range(8):
                band(kT_sb[et][po:po + 64, i * 128:(i + 1) * 128], detp_sb,
                     ek_stage, i, on_act=(i % 8 < 3))
                yield
            nc.gpsimd.dma_start(
                out=bass.AP(tensor=ekd[h], offset=896,
                            ap=[[WD, 128], [128 * WD - 128, 8], [1, 1152]]),
                in_=ek_stage)
            yield

        def emit_reads(h):
            # critical transposes issue ahead of the slack-side ek reads
            rels, ekts = [], []
            for rt in range(8):
                rel = rel_pool.tile([128, S], BF16, name="rel", tag="rel")
                nc.sync.dma_start_transpose(
                    out=rel,
                    in_=bass.AP(tensor=eqr[h], offset=1023 + rt * 128,
                                ap=[[WD - 1, 1024], [1, 128]]))
                rels.append(rel)
            for rt in range(8):
                ekt = ekt_pool.tile([128, S], FP8, name="ekt", tag="ekt")
                nc.sync.dma_start(
                    out=ekt,
                    in_=bass.AP(tensor=ekd[h], offset=(WD - 1) * rt * 128 + 1023,
                                ap=[[WD - 1, 128], [1, 1024]]))
                ekts.append(ekt)
            return rels, ekts

        out16_store = {}

        def score_pv_steps(h, reads):
            """Generator: 8 score steps then 8 PV steps for head h."""
            et, po = h // 2, 64 * (h % 2)
            rels, ekts = reads
            expt = []
            for rt in range(8):
                r0 = rt * 128
                rel, ekt = rels[rt], ekts[rt]
                sc = [psc.tile([128, 512], F32, name="sc", tag="sc")
                      for _ in range(2)]
                for nh in range(2):
                    nc.tensor.matmul(
                        sc[nh],
                        kT_sb[et][po:po + 64, r0:r0 + 128],
                        qT_sb[et][po:po + 64, nh * 512:(nh + 1) * 512],
                        start=True, stop=False)
                for nh in range(2):
                    nc.tensor.matmul(
                        sc[nh], id_sb, rel[:, nh * 512:(nh + 1) * 512],
                        start=False, stop=False)
                for nh in range(2):
                    nc.tensor.matmul(
                        sc[nh], idf8_sb, ekt[:, nh * 512:(nh + 1) * 512],
                        start=False, stop=True)
                ex = expt_pool.tile([128, S], BF16, name="ex", tag="ex")
                for nh in range(2):
                    nc.scalar.activation(out=ex[:, nh * 512:(nh + 1) * 512],
                                         in_=sc[nh], func=EXP,
                                         scale=1.0 / math.sqrt(HD),
                                         bias=mask_sb[:, rt:rt + 1])
                expt.append(ex)
                yield
            ctxden = small.tile([128, 8, 65], F32, name="ctxden", tag="ctxden",
                                bufs=2)
            if h % 2 == 0:
                out16_store[0] = small.tile([128, 8, 128], F32, name="out16",
                                            tag="out16", bufs=1)
            out16 = out16_store[0]
            oc = (h % 2) * 64
            for lt in range(8):
                cx = pctx.tile([128, 65], F32, name="cx", tag="cx")
                for rt in range(8):
                    nc.tensor.matmul(cx, expt[rt][:, lt * 128:(lt + 1) * 128],
                                     v_sb[rt][:, h, :],
                                     start=(rt == 0), stop=(rt == 7))
                # single fast copy releases the PSUM bank; divide later
                nc.vector.tensor_copy(ctxden[:, lt, :], cx)
                yield
            rc8 = small.tile([128, 8], F32, name="rc8", tag="rc8", bufs=2)
            nc.vector.reciprocal(rc8, ctxden[:, :, 64])
            for lt in range(8):
                nc.vector.tensor_scalar_mul(
                    out16[:, lt, oc:oc + 64], ctxden[:, lt, 0:64],
                    rc8[:, lt:lt + 1])
            if h % 2 == 1:
                # one write per head pair: 512B contiguous rows, full DMA rate
                nc.gpsimd.dma_start(
                    out=bass.AP(tensor=out_t, offset=(h // 2) * 128,
                                ap=[[E, 128], [E * 128, 8], [1, 128]]),
                    in_=out16)
            yield

        # depth-3 pipeline, interleaved emission so engine priority order
        # alternates: bands(h) | reads(h-1) | scores+PV(h-2)
        def exhaust(g):
            if g is not None:
                for _ in g:
                    pass

        exhaust(proj_qk_steps(0))
        extras = {0: [proj_qk_steps(1), proj_v_steps(0, 4)],
                  1: [proj_qk_steps(2), proj_v_steps(4, 8)],
                  2: [proj_qk_steps(3)]}
        reads_of = {}
        for h in range(NHC + 2):
            if 1 <= h <= NHC:
                reads_of[h - 1] = emit_reads(h - 1)
            gens = list(extras.get(h, []))
            if h < NHC:
                gens.append(band_steps(h))
            if h >= 2:
                gens.append(score_pv_steps(h - 2, reads_of.pop(h - 2)))
            while gens:
                for g in list(gens):
                    try:
                        next(g)
                    except StopIteration:
                        gens.remove(g)

    nc.compile()
    return nc


_NC_CACHE = {}
LAST_RESULT = None


def kernel(hidden_states, attention_mask, Wq, bq, Wk, bk, Wv, bv, dist_emb):
    hidden_states = np.asarray(hidden_states, np.float32)
    attention_mask = np.asarray(attention_mask, np.float32)
    Wq, bq = np.asarray(Wq, np.float32), np.asarray(bq, np.float32)
    Wk, bk = np.asarray(Wk, np.float32), np.asarray(bk, np.float32)
    Wv, bv = np.asarray(Wv, np.float32), np.asarray(bv, np.float32)
    dist_emb = np.asarray(dist_emb, np.float32)
    bf = mybir.dt.np(BF16)

    deT = dist_emb.T  # [64, 2047]
    deTP = np.zeros((128, WD), np.float32)
    deTP[0:64, :2047] = deT
    deTP[64:128, :2047] = deT
    deTR = np.zeros((128, WD), np.float32)
    deTR[0:64, :2047] = deT[:, ::-1]
    deTR[64:128, :2047] = deT[:, ::-1]
    ident = np.eye(128).astype(bf)

    if "nc" not in _NC_CACHE:
        _NC_CACHE["nc"] = build_nc()
    nc = _NC_CACHE["nc"]

    in_maps = []
    for c in range(8):
        b, g = c // 2, c % 2
        esl = slice(g * E, (g + 1) * E)
        mk = attention_mask[b, 0, 0, :].astype(np.float32)
        in_maps.append({
            "hT": np.ascontiguousarray(hidden_states[b].T).astype(bf),
            "wqT": np.ascontiguousarray(Wq[esl, :].T).astype(bf),
            "wkT": np.ascontiguousarray(Wk[esl, :].T).astype(bf),
            "wvT": np.ascontiguousarray(Wv[esl, :].T).astype(bf),
            "bqc": np.ascontiguousarray(bq[esl].reshape(4, 128).T),
            "bkc": np.ascontiguousarray(bk[esl].reshape(4, 128).T),
            "bvr": np.ascontiguousarray(bv[esl][None, :]).astype(bf),
            "deTR": deTR.astype(bf), "deTP": deTP.astype(bf),
            "maskc": np.ascontiguousarray(mk.reshape(8, 128).T),
            "ident": ident,
        })
    import os as _os
    res = run_bass_kernel_spmd(nc, in_maps, core_ids=list(range(8)),
                               trace=bool(_os.environ.get("KTRACE")),
                               tmpdir=_os.environ.get("KTRACE_DIR") or None)
    global LAST_RESULT
    LAST_RESULT = res
    out = np.empty((B, S, D), np.float32)
    for c in range(8):
        b, g = c // 2, c % 2
        out[b, :, g * E:(g + 1) * E] = res.results[c]["out"]
    return out
